# revision 10
# baseline (speedup 1.0000x reference)
"""Trainium2 Bass kernel for ExampleGNN (2-layer GCN + global_add_pool + head).

Self-contained: accepts FULL inputs, shards across 8 NeuronCores internally,
returns the FULL [64, 32] log-softmax output.

Sharding: nodes (and their incident in-edges) are partitioned across 8 cores
with a degree-balancing permutation (node relabeling is internal; pooling is
order-invariant). 128x128 weights replicated. One AllGather shares layer-1
activations between layers; one AllReduce combines pooled partials.

Per-core pipeline (per layer):
  - edges grouped by destination 512-node block, padded to 128-edge tiles
    (tile structure uniform across cores so one SPMD program serves all 8)
  - dma_gather pulls h[src] rows (512B) from DRAM, <=1024 rows per call
  - one-hot B[e, n] = (dstloc_e == n) * norm_e built in one DVE tensor_scalar
  - PE accumulates aggT[f, n] += gathered^T @ B into PSUM per block
  - h = relu(aggT^T @ W + b) in 128-node chunks (bias via K=1 matmul) + ACT
  - layer 2 also accumulates pooled[g, f] via one-hot batch matmul
"""
import numpy as np

import concourse.bacc as bacc
import concourse.mybir as mybir
import concourse.tile as tile

CORES = 8
N = 50000
D = 128
DOUT = 32
G = 64
NPC = N // CORES           # 6250 nodes per core
BLK = 256                  # aggregation block (PSUM free dim)
NBLK = (NPC + BLK - 1) // BLK   # 13 blocks (last has 106 nodes)
GHALF = 25000              # gather-table split for int16 indices
MAX_SEG_TILES = 8          # cap per dma_gather call (2048-idx calls crash HW)
ABLATE = "full"            # "gather" | "compute" | "full" (ablation)
B_BUFS = 8
AGG_BUFS = 2
GATHER_BUFS = 4
BF16 = False               # gather table / B matrices in bf16 (2x engine rates)
QUEUES = 1                 # SWDGE queues for parallel dma_gather streams
DBG = False                # extra debug outputs (h1 slice, pooled partial)
NODIAG = False             # debug: skip the self-loop diagonal term
IDX_ALIGN = 16             # segment row-count alignment (128 for bf16 tables)

f32 = mybir.dt.float32
bf16 = mybir.dt.bfloat16
i16 = mybir.dt.int16


# ---------------------------------------------------------------- host prep --

def _wrap_idxs(idx):
    """[n] -> [128, n//16] int16 wrapped layout (16-partition groups,
    replicated for the 8 gpsimd cores)."""
    n = len(idx)
    t = np.asarray(idx, dtype=np.int16).reshape(n // 16, 16).T
    return np.ascontiguousarray(np.tile(t, (8, 1)))


def prep(edge_index, batch):
    """Host-side index prep. Returns (structure, per_core arrays, node perm).

    perm[old_id] = new_id; new ids are contiguous per (core, block) with
    in-degree-balanced assignment (LPT) so per-block edge counts match
    across cores (less tile padding in the shared SPMD program).
    """
    src_o = np.asarray(edge_index[0], dtype=np.int64)
    dst_o = np.asarray(edge_index[1], dtype=np.int64)
    loops = np.arange(N, dtype=np.int64)
    deg = (np.bincount(dst_o, minlength=N) + 1).astype(np.float32)
    dinv = (1.0 / np.sqrt(deg)).astype(np.float32)
    # self-loops handled as a diagonal term (dinv^2 * h) on the compute side;
    # only real edges go through the gather path
    src_all = src_o
    dst_all = dst_o
    norm = (dinv[src_all] * dinv[dst_all]).astype(np.float32)

    # ---- LPT balance: assign nodes (by desc in-degree) to 8*NBLK bins
    nbins = CORES * NBLK
    cap = np.full(nbins, BLK, dtype=np.int64)
    cap[NBLK - 1::NBLK] = NPC - (NBLK - 1) * BLK   # last block per core: 106
    order = np.argsort(-deg, kind="stable")
    load = np.zeros(nbins, dtype=np.float64)
    fill = np.zeros(nbins, dtype=np.int64)
    perm = np.empty(N, dtype=np.int64)
    import heapq
    heap = [(0.0, int(b)) for b in range(nbins)]
    heapq.heapify(heap)
    for nid in order:
        while True:
            l, b = heapq.heappop(heap)
            if fill[b] < cap[b]:
                break
        c, blk_i = divmod(b, NBLK)
        perm[nid] = c * NPC + blk_i * BLK + fill[b]
        fill[b] += 1
        l += float(deg[nid])
        if fill[b] < cap[b]:
            heapq.heappush(heap, (l, b))

    src = perm[src_all]
    dst = perm[dst_all]

    core = dst // NPC
    dstloc = dst - core * NPC
    blk = dstloc // BLK
    dsub = (dstloc % BLK).astype(np.float32)
    grp = (src >= GHALF).astype(np.int64)
    idx16 = (src - grp * GHALF).astype(np.int16)

    so = np.lexsort((grp, blk, core))
    idx_s, core_s = idx16[so], core[so]
    blk_s, grp_s, dsub_s, norm_s = blk[so], grp[so], dsub[so], norm[so]

    cnt = np.zeros((CORES, NBLK, 2), dtype=np.int64)
    np.add.at(cnt, (core_s, blk_s, grp_s), 1)
    ccap = -(-cnt.max(axis=0) // IDX_ALIGN) * IDX_ALIGN   # [NBLK, 2]

    seg_tiles = []            # ordered (b, g, nidx); nidx%16==0, <=MAX*128
    for b in range(NBLK):
        for g in range(2):
            r = int(ccap[b, g])
            while r > MAX_SEG_TILES * 128:
                seg_tiles.append((b, g, MAX_SEG_TILES * 128))
                r -= MAX_SEG_TILES * 128
            if r > 0:
                seg_tiles.append((b, g, r))
    ttot = sum(-(-s[2] // 128) for s in seg_tiles)

    starts = np.cumsum(np.concatenate([[0], cnt.reshape(-1)]))[:-1].reshape(cnt.shape)
    itot = sum(s[2] for s in seg_tiles)        # gathered rows (16-aligned)
    per_core = []
    for c in range(CORES):
        idx_flat = np.zeros(itot, dtype=np.int16)
        dsub_flat = np.zeros(ttot * 128, dtype=np.float32)
        norm_flat = np.zeros(ttot * 128, dtype=np.float32)
        pos = 0    # in gathered-row space (16-aligned per segment)
        tpos = 0   # in tile space (dsub/nrm tile-major)
        used = {}
        for (b, g, ni) in seg_tiles:
            u = used.get((b, g), 0)
            take = min(ni, cnt[c, b, g] - u)
            if take > 0:
                sl = slice(starts[c, b, g] + u, starts[c, b, g] + u + take)
                idx_flat[pos:pos + take] = idx_s[sl]
                dsub_flat[tpos:tpos + take] = dsub_s[sl]
                norm_flat[tpos:tpos + take] = norm_s[sl]
                used[(b, g)] = u + take
            pos += ni
            tpos += -(-ni // 128) * 128
        per_core.append({
            "idx": _wrap_idxs(idx_flat),
            "dsub": np.ascontiguousarray(dsub_flat.reshape(ttot, 128).T),
            "nrm": np.ascontiguousarray(norm_flat.reshape(ttot, 128).T),
        })

    # batch id / dinv^2 per new node id, [128, n_chunks] column per chunk
    batch = np.asarray(batch, dtype=np.int64)
    batch_new = np.zeros(N, dtype=np.float32)
    batch_new[perm] = batch.astype(np.float32)
    dinv2_new = np.zeros(N, dtype=np.float32)
    dinv2_new[perm] = dinv * dinv
    nchunk = (NPC + 127) // 128
    for c in range(CORES):
        bl = np.zeros(nchunk * 128, dtype=np.float32)
        bl[:NPC] = batch_new[c * NPC:(c + 1) * NPC]
        per_core[c]["bloc"] = np.ascontiguousarray(bl.reshape(nchunk, 128).T)
        d2 = np.zeros(nchunk * 128, dtype=np.float32)
        d2[:NPC] = dinv2_new[c * NPC:(c + 1) * NPC]
        per_core[c]["dinv2"] = np.ascontiguousarray(d2.reshape(nchunk, 128).T)

    struct = {"seg_tiles": seg_tiles, "ttot": ttot, "itot": itot}
    return struct, per_core, perm


def ml_dtypes_bf16():
    import ml_dtypes
    return ml_dtypes.bfloat16


def make_consts():
    iota512 = np.tile(np.arange(BLK, dtype=np.float32), (128, 1))
    iota64 = np.tile(np.arange(64, dtype=np.float32), (128, 1))
    ident = np.eye(128, dtype=np.float32)
    ones = np.ones((1, 128), dtype=np.float32)
    return {"iota512": iota512, "iota64": iota64, "ident": ident, "ones": ones}


# ------------------------------------------------------------------ program --

def build(struct, timed_reps=None, gather_bufs=None):
    if gather_bufs is None:
        gather_bufs = GATHER_BUFS
    seg_tiles = struct["seg_tiles"]
    ttot = struct["ttot"]
    timed = timed_reps is not None
    nchunk = (NPC + 127) // 128

    nc = bacc.Bacc("TRN2", target_bir_lowering=False, debug=False,
                   num_devices=CORES, num_swdge_queues=QUEUES)

    itot = struct["itot"]
    gdt = bf16 if BF16 else f32
    # single gather-table input: f32 "x" or bf16 "xg" (an unused ExternalInput
    # would be elided by the compiler and shift NEFF input bindings)
    if BF16:
        xg = nc.dram_tensor("xg", [N, D], gdt, kind="ExternalInput")
    else:
        xg = nc.dram_tensor("x", [N, D], f32, kind="ExternalInput")
    xloc = nc.dram_tensor("xloc", [nchunk * 128, D], gdt, kind="ExternalInput")
    idx = nc.dram_tensor("idx", [128, itot // 16], i16, kind="ExternalInput")
    dsub = nc.dram_tensor("dsub", [128, ttot], f32, kind="ExternalInput")
    nrm = nc.dram_tensor("nrm", [128, ttot], f32, kind="ExternalInput")
    bloc = nc.dram_tensor("bloc", [128, nchunk], f32, kind="ExternalInput")
    dinv2 = nc.dram_tensor("dinv2", [128, nchunk], f32, kind="ExternalInput")
    w1 = nc.dram_tensor("w1", [D, D], f32, kind="ExternalInput")
    w2 = nc.dram_tensor("w2", [D, D], f32, kind="ExternalInput")
    wh = nc.dram_tensor("wh", [D, DOUT], f32, kind="ExternalInput")
    b1 = nc.dram_tensor("b1", [1, D], f32, kind="ExternalInput")
    b2 = nc.dram_tensor("b2", [1, D], f32, kind="ExternalInput")
    bh = nc.dram_tensor("bh", [1, DOUT], f32, kind="ExternalInput")
    iota512 = nc.dram_tensor("iota512", [128, BLK], gdt, kind="ExternalInput")
    iota64 = nc.dram_tensor("iota64", [128, 64], f32, kind="ExternalInput")
    ident = nc.dram_tensor("ident", [128, 128], f32, kind="ExternalInput")
    ones = nc.dram_tensor("ones", [1, 128], f32, kind="ExternalInput")
    out = nc.dram_tensor("out", [G, DOUT], f32, kind="ExternalOutput")
    if DBG:
        dbg_h1 = nc.dram_tensor("dbg_h1", [2048, D], f32, kind="ExternalOutput")
        dbg_h1b = nc.dram_tensor("dbg_h1b", [1024, D], f32, kind="ExternalOutput")
        dbg_agg = nc.dram_tensor("dbg_agg", [D, BLK], f32, kind="ExternalOutput")
        dbg_pool = nc.dram_tensor("dbg_pool", [G, D], f32, kind="ExternalOutput")

    with tile.TileContext(nc) as tc:
        with tc.tile_pool(name="const", bufs=1) as cp, \
             tc.tile_pool(name="gat", bufs=gather_bufs) as gp, \
             tc.tile_pool(name="bt", bufs=B_BUFS) as bp, \
             tc.tile_pool(name="hs", bufs=3) as hp, \
             tc.tile_pool(name="agg", bufs=AGG_BUFS, space="PSUM") as aggp, \
             tc.tile_pool(name="hps", bufs=2, space="PSUM") as hpsp, \
             tc.tile_pool(name="mps", bufs=2, space="PSUM") as mpsp, \
             tc.tile_pool(name="hd", bufs=1, space="PSUM") as hdp, \
             tc.tile_pool(name="dram", bufs=1, space="DRAM") as dp:

            idx_sb = cp.tile([128, itot // 16], i16)
            nc.sync.dma_start(idx_sb[:], idx[:])
            dsub_sb = cp.tile([128, ttot], f32)
            nc.sync.dma_start(dsub_sb[:], dsub[:])
            nrm_sb = cp.tile([128, ttot], f32)
            nc.sync.dma_start(nrm_sb[:], nrm[:])
            bloc_sb = cp.tile([128, nchunk], f32)
            nc.sync.dma_start(bloc_sb[:], bloc[:])
            dinv2_sb = cp.tile([128, nchunk], f32)
            nc.sync.dma_start(dinv2_sb[:], dinv2[:])
            w1_sb = cp.tile([D, D], f32)
            nc.sync.dma_start(w1_sb[:], w1[:])
            w2_sb = cp.tile([D, D], f32)
            nc.sync.dma_start(w2_sb[:], w2[:])
            wh_sb = cp.tile([D, DOUT], f32)
            nc.sync.dma_start(wh_sb[:], wh[:])
            b1_sb = cp.tile([1, D], f32)
            nc.sync.dma_start(b1_sb[:], b1[:])
            b2_sb = cp.tile([1, D], f32)
            nc.sync.dma_start(b2_sb[:], b2[:])
            bh_sb = cp.tile([1, DOUT], f32)
            nc.sync.dma_start(bh_sb[:], bh[:])
            io512_sb = cp.tile([128, BLK], gdt)
            nc.sync.dma_start(io512_sb[:], iota512[:])
            io64_sb = cp.tile([128, 64], f32)
            nc.sync.dma_start(io64_sb[:], iota64[:])
            id_sb = cp.tile([128, 128], f32)
            nc.sync.dma_start(id_sb[:], ident[:])
            ones_sb = cp.tile([1, 128], f32)
            nc.sync.dma_start(ones_sb[:], ones[:])

            pool_acc = cp.tile([G, D], f32)

            h1_bounce = dp.tile([NPC, D], bf16 if BF16 else f32)
            nrep = timed_reps if timed else 1
            h1_fulls = [dp.tile([N, D], bf16 if BF16 else f32,
                                addr_space="Shared", name=f"h1_full_{r}")
                        for r in range(nrep)]
            pool_ins = [dp.tile([G, D], f32, name=f"pool_in_{r}")
                        for r in range(nrep)]
            pool_outs = [dp.tile([G, D], f32, addr_space="Shared",
                                 name=f"pool_out_{r}") for r in range(nrep)]
            h1_full, pool_in, pool_out = h1_fulls[0], pool_ins[0], pool_outs[0]

            def do_layer(layer, table, w_sb, b_sb, diag_tab):
                t = 0      # tile index (dsub/nrm columns)
                ipos = 0   # gathered-row index (16-aligned)
                agg_ps = None
                seen = set()
                for si, (b, g, ni) in enumerate(seg_tiles):
                    nt = -(-ni // 128)
                    if b not in seen:
                        seen.add(b)
                        first_of_blk = True
                        agg_ps = aggp.tile([128, BLK], f32, tag="agg")
                    else:
                        first_of_blk = False
                    gat = gp.tile([128, MAX_SEG_TILES, D], gdt, tag="gat")
                    if ABLATE in ("full", "gather"):
                        nc.gpsimd.dma_gather(
                            gat[:, :nt, :],
                            table[g * GHALF:(g + 1) * GHALF, :],
                            idx_sb[:, ipos // 16:(ipos + ni) // 16],
                            ni, ni, D, single_packet=False,
                            queue_num=si % QUEUES)
                    else:
                        r0 = (t * 128) % (N - MAX_SEG_TILES * 128)
                        nc.sync.dma_start(
                            gat[:, :nt, :],
                            table[r0:r0 + nt * 128, :].rearrange(
                                "(a p) d -> p a d", p=128))
                    last_seg_of_blk = (
                        si + 1 == len(seg_tiles) or seg_tiles[si + 1][0] != b)
                    if ABLATE == "gather":
                        nc.vector.tensor_add(pool_acc[:1, :1], pool_acc[:1, :1],
                                             gat[:1, 0, :1])
                        if first_of_blk:
                            nc.tensor.matmul(agg_ps[:], lhsT=gat[:, 0, :],
                                             rhs=io512_sb[:],
                                             start=True, stop=True)
                    elif ABLATE == "pe":
                        for k in range(nt):
                            nc.tensor.matmul(
                                agg_ps[:], lhsT=gat[:, k, :], rhs=io512_sb[:],
                                start=(first_of_blk and k == 0),
                                stop=(last_seg_of_blk and k == nt - 1))
                    elif ABLATE == "pe128":
                        for k in range(nt):
                            nc.tensor.matmul(
                                agg_ps[:, :128], lhsT=gat[:, k, :],
                                rhs=io512_sb[:, :128],
                                start=(first_of_blk and k == 0),
                                stop=(last_seg_of_blk and k == nt - 1))
                    elif ABLATE == "dve":
                        for k in range(nt):
                            bmat = bp.tile([128, BLK], gdt, tag="B")
                            nc.vector.tensor_scalar(
                                out=bmat[:], in0=io512_sb[:],
                                scalar1=dsub_sb[:, t + k:t + k + 1],
                                scalar2=nrm_sb[:, t + k:t + k + 1],
                                op0=mybir.AluOpType.is_equal,
                                op1=mybir.AluOpType.mult)
                            nc.vector.tensor_add(pool_acc[:1, :1],
                                                 pool_acc[:1, :1], bmat[:1, :1])
                        if first_of_blk:
                            nc.tensor.matmul(agg_ps[:], lhsT=gat[:, 0, :],
                                             rhs=io512_sb[:],
                                             start=True, stop=True)
                    else:
                        for k in range(nt):
                            kk = min(128, ni - k * 128)
                            bmat = bp.tile([128, BLK], gdt, tag="B")
                            nc.vector.tensor_scalar(
                                out=bmat[:kk, :], in0=io512_sb[:kk, :],
                                scalar1=dsub_sb[:kk, t + k:t + k + 1],
                                scalar2=nrm_sb[:kk, t + k:t + k + 1],
                                op0=mybir.AluOpType.is_equal,
                                op1=mybir.AluOpType.mult)
                            nc.tensor.matmul(
                                agg_ps[:], lhsT=gat[:kk, k, :], rhs=bmat[:kk, :],
                                start=(first_of_blk and k == 0),
                                stop=(last_seg_of_blk and k == nt - 1))
                    t += nt
                    ipos += ni
                    if not last_seg_of_blk:
                        continue
                    # block b complete: per 128-chunk h = relu(aggT^T W + b)
                    aggt_sb = hp.tile([128, BLK], f32, tag="aggt")
                    nc.vector.tensor_copy(out=aggt_sb[:], in_=agg_ps[:])
                    # diagonal (self-loop) term: aggT[:, n] += dinv2_n * h_loc[n]
                    bw0 = BLK if b < NBLK - 1 else NPC - (NBLK - 1) * BLK
                    nck = (bw0 + 127) // 128
                    for cki in range(nck if not NODIAG else 0):
                        w = min(128, bw0 - cki * 128)
                        ck = b * (BLK // 128) + cki
                        r0 = b * BLK + cki * 128
                        dloc = gp.tile([128, D], gdt, tag="dloc")
                        nc.sync.dma_start(dloc[:w, :], diag_tab[r0:r0 + w, :])
                        dscl = bp.tile([128, D], f32, tag="dscl")
                        nc.vector.tensor_scalar(
                            out=dscl[:w, :], in0=dloc[:w, :],
                            scalar1=dinv2_sb[:w, ck:ck + 1], scalar2=None,
                            op0=mybir.AluOpType.mult)
                        dps = hdp.tile([128, 128], f32, tag="dps")
                        nc.tensor.transpose(dps[:, :w], dscl[:w, :],
                                            id_sb[:w, :w])
                        nc.vector.tensor_add(
                            aggt_sb[:, cki * 128:cki * 128 + w],
                            aggt_sb[:, cki * 128:cki * 128 + w],
                            dps[:, :w])
                    if DBG and layer == 1 and b == 0:
                        nc.sync.dma_start(dbg_agg[:, :], aggt_sb[:])
                    bw = BLK if b < NBLK - 1 else NPC - (NBLK - 1) * BLK
                    for cki in range((bw + 127) // 128):
                        w = min(128, bw - cki * 128)
                        ck = b * (BLK // 128) + cki
                        h_ps = hpsp.tile([128, 128], f32, tag="hps")
                        nc.tensor.matmul(
                            h_ps[:], lhsT=aggt_sb[:, cki * 128:cki * 128 + 128],
                            rhs=w_sb[:], start=True, stop=False)
                        nc.tensor.matmul(h_ps[:], lhsT=ones_sb[:, :128],
                                         rhs=b_sb[:], start=False, stop=True)
                        h_sb = hp.tile([128, 128],
                                       gdt if layer == 1 else f32, tag="h")
                        nc.scalar.activation(h_sb[:], h_ps[:],
                                             mybir.ActivationFunctionType.Relu)
                        r0 = b * BLK + cki * 128
                        if layer == 1:
                            nc.sync.dma_start(
                                h1_bounce[r0:r0 + w, :], h_sb[:w, :])
                        else:
                            pmat = bp.tile([128, 64], f32, tag="P")
                            nc.vector.tensor_scalar(
                                out=pmat[:], in0=io64_sb[:],
                                scalar1=bloc_sb[:, ck:ck + 1], scalar2=None,
                                op0=mybir.AluOpType.is_equal)
                            m_ps = mpsp.tile([G, D], f32, tag="mps")
                            nc.tensor.matmul(m_ps[:], lhsT=pmat[:w, :],
                                             rhs=h_sb[:w, :],
                                             start=True, stop=True)
                            nc.vector.tensor_add(pool_acc[:], pool_acc[:],
                                                 m_ps[:])

            def head():
                pt_ps = hdp.tile([D, G], f32, tag="hd")
                nc.tensor.transpose(pt_ps[:], pool_acc[:], id_sb[:G, :G])
                pt_sb = hp.tile([D, G], f32, tag="pt")
                nc.vector.tensor_copy(out=pt_sb[:], in_=pt_ps[:])
                lg_ps = hdp.tile([G, DOUT], f32, tag="hd")
                nc.tensor.matmul(lg_ps[:], lhsT=pt_sb[:], rhs=wh_sb[:],
                                 start=True, stop=False)
                nc.tensor.matmul(lg_ps[:], lhsT=ones_sb[:, :G], rhs=bh_sb[:],
                                 start=False, stop=True)
                lg_sb = hp.tile([G, DOUT], f32, tag="lg")
                nc.vector.tensor_copy(out=lg_sb[:], in_=lg_ps[:])
                mx = hp.tile([G, 1], f32, tag="mx")
                nc.vector.reduce_max(mx[:], lg_sb[:], axis=mybir.AxisListType.X)
                nc.vector.tensor_scalar(out=lg_sb[:], in0=lg_sb[:],
                                        scalar1=mx[:], scalar2=None,
                                        op0=mybir.AluOpType.subtract)
                ex = hp.tile([G, DOUT], f32, tag="ex")
                nc.scalar.activation(ex[:], lg_sb[:],
                                     mybir.ActivationFunctionType.Exp)
                sm = hp.tile([G, 1], f32, tag="sm")
                nc.vector.reduce_sum(sm[:], ex[:], axis=mybir.AxisListType.X)
                ls = hp.tile([G, 1], f32, tag="ls")
                nc.scalar.activation(ls[:], sm[:],
                                     mybir.ActivationFunctionType.Ln)
                nc.vector.tensor_scalar(out=lg_sb[:], in0=lg_sb[:],
                                        scalar1=ls[:], scalar2=None,
                                        op0=mybir.AluOpType.subtract)
                nc.sync.dma_start(out[:, :], lg_sb[:])

            def whole(rep):
                nc.vector.memset(pool_acc[:], 0.0)
                do_layer(1, xg, w1_sb, b1_sb, xloc)
                nc.gpsimd.collective_compute(
                    "AllGather", mybir.AluOpType.bypass,
                    replica_groups=[list(range(CORES))],
                    ins=[h1_bounce[:, :].opt()],
                    outs=[h1_fulls[rep][:, :].opt()])
                do_layer(2, h1_fulls[rep], w2_sb, b2_sb, h1_bounce)
                nc.sync.dma_start(pool_ins[rep][:, :], pool_acc[:])
                nc.gpsimd.collective_compute(
                    "AllReduce", mybir.AluOpType.add,
                    replica_groups=[list(range(CORES))],
                    ins=[pool_ins[rep][:, :].opt()],
                    outs=[pool_outs[rep][:, :].opt()])
                nc.sync.dma_start(pool_acc[:], pool_outs[rep][:, :])
                head()

            if timed:
                for _rep in range(timed_reps):
                    whole(_rep)
            else:
                nc.vector.memset(pool_acc[:], 0.0)
                do_layer(1, xg, w1_sb, b1_sb, xloc)
                nc.gpsimd.collective_compute(
                    "AllGather", mybir.AluOpType.bypass,
                    replica_groups=[list(range(CORES))],
                    ins=[h1_bounce[:, :].opt()], outs=[h1_full[:, :].opt()])
                if DBG:
                    for dk in range(8):
                        btile = hp.tile([128, D], gdt, tag="dbgb")
                        nc.sync.dma_start(
                            btile[:], h1_bounce[dk * 128:(dk + 1) * 128, :])
                        btf = hp.tile([128, D], f32, tag="dbgbf")
                        nc.vector.tensor_copy(out=btf[:], in_=btile[:])
                        nc.sync.dma_start(
                            dbg_h1b[dk * 128:(dk + 1) * 128, :], btf[:])
                    for dk in range(16):
                        dtile = hp.tile([128, D], gdt, tag="dbgt")
                        nc.sync.dma_start(
                            dtile[:], h1_full[dk * 128:(dk + 1) * 128, :])
                        dtf = hp.tile([128, D], f32, tag="dbgf")
                        nc.vector.tensor_copy(out=dtf[:], in_=dtile[:])
                        nc.sync.dma_start(
                            dbg_h1[dk * 128:(dk + 1) * 128, :], dtf[:])
                    nc.sync.dma_start(dbg_pool[:, :], pool_acc[:])
                do_layer(2, h1_full, w2_sb, b2_sb, h1_bounce)
                nc.sync.dma_start(pool_in[:, :], pool_acc[:])
                nc.gpsimd.collective_compute(
                    "AllReduce", mybir.AluOpType.add,
                    replica_groups=[list(range(CORES))],
                    ins=[pool_in[:, :].opt()], outs=[pool_out[:, :].opt()])
                nc.sync.dma_start(pool_acc[:], pool_out[:, :])
                head()

    nc.compile()
    return nc


def make_in_maps(inputs, per_core, perm):
    consts = make_consts()
    x = np.asarray(inputs["x"], dtype=np.float32)
    x_perm = np.empty_like(x)
    x_perm[perm] = x
    base = {}
    if BF16:
        import ml_dtypes
        base["xg"] = np.ascontiguousarray(x_perm.astype(ml_dtypes.bfloat16))
        base["iota512"] = consts["iota512"].astype(ml_dtypes.bfloat16)
    else:
        base["x"] = np.ascontiguousarray(x_perm)
    gnp = x_perm.astype(ml_dtypes_bf16()) if BF16 else x_perm
    base.update({
        "w1": np.asarray(inputs["W1"], dtype=np.float32),
        "w2": np.asarray(inputs["W2"], dtype=np.float32),
        "wh": np.asarray(inputs["Wh"], dtype=np.float32),
        "b1": np.asarray(inputs["b1"], dtype=np.float32).reshape(1, D),
        "b2": np.asarray(inputs["b2"], dtype=np.float32).reshape(1, D),
        "bh": np.asarray(inputs["bh"], dtype=np.float32).reshape(1, DOUT),
        **{k: v for k, v in consts.items() if k not in base},
    })
    in_maps = []
    for c in range(CORES):
        m = dict(base)
        for k in ("idx", "dsub", "nrm", "bloc", "dinv2"):
            m[k] = per_core[c][k]
        nchunk = (NPC + 127) // 128
        xl = np.zeros((nchunk * 128, D), dtype=gnp.dtype)
        xl[:NPC] = gnp[c * NPC:(c + 1) * NPC]
        m["xloc"] = xl

        in_maps.append(m)
    return in_maps


def kernel(**inputs) -> np.ndarray:
    struct, per_core, perm = prep(inputs["edge_index"], inputs["batch"])
    nc = build(struct)
    in_maps = make_in_maps(inputs, per_core, perm)
    from concourse.bass_utils import run_bass_kernel_spmd
    res = run_bass_kernel_spmd(nc, in_maps, core_ids=list(range(CORES)))
    return np.asarray(res.results[0]["out"], dtype=np.float32)


if __name__ == "__main__":
    import reference
    inputs = reference.setup_inputs()
    got = kernel(**{k: np.asarray(v) for k, v in inputs.items()})
    print(got[:2])



# revision 15
# speedup vs baseline: 1.9593x; 1.9593x over previous
"""Trainium2 Bass kernel for ExampleGNN (2-layer GCN + global_add_pool + head).

Self-contained: accepts FULL inputs, shards across 8 NeuronCores internally,
returns the FULL [64, 32] log-softmax output.

Sharding: nodes (and their incident in-edges) are partitioned across 8 cores
with a degree-balancing permutation (node relabeling is internal; pooling is
order-invariant). 128x128 weights replicated. One AllGather shares layer-1
activations between layers; one AllReduce combines pooled partials.

v2 pipeline (per core, per layer) — no DVE in the hot path:
  - norm factored: gather table rows pre-scaled by dinv[src] on host;
    dinv[dst] applied via the post-matmul activation scale (relu(dinv^2 y)),
    bias kept exact with a rdinv-valued K=1 bias matmul
  - one-hot B tiles are therefore pure 0/1: host-precomputed fp8, streamed
    as contiguous DMA, fed straight to PE (lhsT bf16 x rhs fp8)
  - dma_gather pulls scaled h[src] rows (256B bf16) from DRAM, 1024/call,
    round-robin over 4 SWDGE queues
  - PE accumulates aggT[f, n] += gathered^T @ B into PSUM per 256-node block;
    the self-loop diagonal term joins the same PSUM bank via a
    transpose-matmul accumulate (start=False)
  - h chunks: PSUM h = aggT^T W (+ rdinv*b), ACT relu with per-node scale
  - layer 2 pools via host-precomputed fp8 one-hot pmat into persistent PSUM
"""
import numpy as np

import concourse.bacc as bacc
import concourse.mybir as mybir
import concourse.tile as tile

CORES = 8
N = 50000
D = 128
DOUT = 32
G = 64
NPC = N // CORES           # 6250 nodes per core
BLK = 256                  # aggregation block (PSUM free dim)
NBLK = (NPC + BLK - 1) // BLK   # 25 blocks (last has 106 nodes)
NCHUNK = (NPC + 127) // 128     # 49 chunks of 128 nodes
GHALF = 25000              # gather-table split for int16 indices
MAX_SEG_TILES = 8          # cap per dma_gather call (2048-idx calls crash HW)
IDX_ALIGN = 128            # segment row-count alignment (bf16 tables need 128)
ABLATE = "full"            # "gather" | "nogather" | "full"
QUEUES = 4                 # SWDGE queues for parallel dma_gather streams
GATHER_BUFS = 8
B_BUFS = 6
AGG_BUFS = 2

f32 = mybir.dt.float32
bf16 = mybir.dt.bfloat16
fp8 = mybir.dt.bfloat16  # one-hot dtype (bf16: mixed-dtype PE runs at low precision w/ fp8)
i16 = mybir.dt.int16


# ---------------------------------------------------------------- host prep --

def _np_dt(dt):
    return mybir.dt.np(dt)


def _wrap_idxs(idx):
    """[n] -> [128, n//16] int16 wrapped layout (16-partition groups,
    replicated for the 8 gpsimd cores)."""
    n = len(idx)
    t = np.asarray(idx, dtype=np.int16).reshape(n // 16, 16).T
    return np.ascontiguousarray(np.tile(t, (8, 1)))


def prep(edge_index, batch):
    """Host-side index prep. Returns (structure, per_core arrays, node perm,
    dinv in old-id space).

    perm[old_id] = new_id; new ids are contiguous per (core, block) with
    in-degree-balanced assignment (LPT) so per-block edge counts match
    across cores (less tile padding in the shared SPMD program).
    """
    src_o = np.asarray(edge_index[0], dtype=np.int64)
    dst_o = np.asarray(edge_index[1], dtype=np.int64)
    deg = (np.bincount(dst_o, minlength=N) + 1).astype(np.float32)
    dinv = (1.0 / np.sqrt(deg)).astype(np.float32)
    # self-loops handled as a diagonal term (dinv * table row) on the PE side;
    # only real edges go through the gather path

    # ---- LPT balance: assign nodes (by desc in-degree) to 8*NBLK bins
    nbins = CORES * NBLK
    cap = np.full(nbins, BLK, dtype=np.int64)
    cap[NBLK - 1::NBLK] = NPC - (NBLK - 1) * BLK   # last block per core: 106
    order = np.argsort(-deg, kind="stable")
    fill = np.zeros(nbins, dtype=np.int64)
    perm = np.empty(N, dtype=np.int64)
    import heapq
    heap = [(0.0, int(b)) for b in range(nbins)]
    heapq.heapify(heap)
    for nid in order:
        while True:
            l, b = heapq.heappop(heap)
            if fill[b] < cap[b]:
                break
        c, blk_i = divmod(b, NBLK)
        perm[nid] = c * NPC + blk_i * BLK + fill[b]
        fill[b] += 1
        l += float(deg[nid])
        if fill[b] < cap[b]:
            heapq.heappush(heap, (l, b))

    src = perm[src_o]
    dst = perm[dst_o]

    core = dst // NPC
    dstloc = dst - core * NPC
    blk = dstloc // BLK
    dsub = (dstloc % BLK).astype(np.int64)
    grp = (src >= GHALF).astype(np.int64)
    idx16 = (src - grp * GHALF).astype(np.int16)

    so = np.lexsort((grp, blk, core))
    idx_s, core_s = idx16[so], core[so]
    blk_s, grp_s, dsub_s = blk[so], grp[so], dsub[so]

    cnt = np.zeros((CORES, NBLK, 2), dtype=np.int64)
    np.add.at(cnt, (core_s, blk_s, grp_s), 1)
    ccap = -(-cnt.max(axis=0) // IDX_ALIGN) * IDX_ALIGN   # [NBLK, 2]

    seg_tiles = []            # ordered (b, g, nidx); nidx%128==0, <=MAX*128
    for b in range(NBLK):
        for g in range(2):
            r = int(ccap[b, g])
            while r > MAX_SEG_TILES * 128:
                seg_tiles.append((b, g, MAX_SEG_TILES * 128))
                r -= MAX_SEG_TILES * 128
            if r > 0:
                seg_tiles.append((b, g, r))
    ttot = sum(-(-s[2] // 128) for s in seg_tiles)

    starts = np.cumsum(np.concatenate([[0], cnt.reshape(-1)]))[:-1].reshape(cnt.shape)
    itot = sum(s[2] for s in seg_tiles)        # gathered rows (128-aligned)
    fp8np = _np_dt(fp8)
    per_core = []
    for c in range(CORES):
        idx_flat = np.zeros(itot, dtype=np.int16)
        bcol = np.full(ttot * 128, -1, dtype=np.int64)  # -1 = pad row
        pos = 0    # in gathered-row space
        tpos = 0   # in tile space (tile-major rows)
        used = {}
        for (b, g, ni) in seg_tiles:
            u = used.get((b, g), 0)
            take = min(ni, cnt[c, b, g] - u)
            if take > 0:
                sl = slice(starts[c, b, g] + u, starts[c, b, g] + u + take)
                idx_flat[pos:pos + take] = idx_s[sl]
                bcol[tpos:tpos + take] = dsub_s[sl]
                used[(b, g)] = u + take
            pos += ni
            tpos += -(-ni // 128) * 128
        # B one-hot: [ttot*128 rows, BLK] -> [128, ttot*BLK] partition-major
        bm = np.zeros((ttot * 128, BLK), dtype=fp8np)
        rr = np.nonzero(bcol >= 0)[0]
        bm[rr, bcol[rr]] = 1.0
        bm = bm.reshape(ttot, 128, BLK).transpose(1, 0, 2).reshape(128, ttot * BLK)
        per_core.append({
            "idx": _wrap_idxs(idx_flat),
            "bmat": np.ascontiguousarray(bm),
        })

    # per new-node-id vectors
    batch = np.asarray(batch, dtype=np.int64)
    batch_new = np.zeros(N, dtype=np.int64)
    batch_new[perm] = batch
    dinv_new = np.zeros(N, dtype=np.float32)
    dinv_new[perm] = dinv
    for c in range(CORES):
        lo, hi = c * NPC, (c + 1) * NPC
        dv = np.zeros(NCHUNK * 128, dtype=np.float32)
        dv[:NPC] = dinv_new[lo:hi]
        per_core[c]["dinvc"] = np.ascontiguousarray(
            dv.reshape(NCHUNK, 128).T)
        per_core[c]["dinv2c"] = np.ascontiguousarray(
            (dv * dv).reshape(NCHUNK, 128).T)
        rd = np.ones(NCHUNK * 128, dtype=np.float32)
        rd[:NPC] = 1.0 / dinv_new[lo:hi]
        per_core[c]["rdinv"] = rd.reshape(1, NCHUNK * 128)
        pm = np.zeros((NCHUNK * 128, G), dtype=fp8np)
        bl = batch_new[lo:hi]
        pm[np.arange(NPC), bl] = 1.0
        pm = pm.reshape(NCHUNK, 128, G).transpose(1, 0, 2).reshape(128, NCHUNK * G)
        per_core[c]["pmat"] = np.ascontiguousarray(pm)

    struct = {"seg_tiles": seg_tiles, "ttot": ttot, "itot": itot}
    return struct, per_core, perm, dinv


def make_consts():
    ident = np.eye(128, dtype=np.float32)
    ones = np.ones((1, 128), dtype=np.float32)
    return {"ident": ident, "ones": ones}


# ------------------------------------------------------------------ program --

def build(struct, timed_reps=None):
    seg_tiles = struct["seg_tiles"]
    ttot = struct["ttot"]
    itot = struct["itot"]
    timed = timed_reps is not None

    nc = bacc.Bacc("TRN2", target_bir_lowering=False, debug=False,
                   num_devices=CORES, num_swdge_queues=QUEUES)

    xg = nc.dram_tensor("xg", [N, D], bf16, kind="ExternalInput")
    xloc = nc.dram_tensor("xloc", [NCHUNK * 128, D], bf16, kind="ExternalInput")
    idx = nc.dram_tensor("idx", [128, itot // 16], i16, kind="ExternalInput")
    bmat = nc.dram_tensor("bmat", [128, ttot * BLK], fp8, kind="ExternalInput")
    pmat = nc.dram_tensor("pmat", [128, NCHUNK * G], fp8, kind="ExternalInput")
    dinvc = nc.dram_tensor("dinvc", [128, NCHUNK], f32, kind="ExternalInput")
    dinv2c = nc.dram_tensor("dinv2c", [128, NCHUNK], f32, kind="ExternalInput")
    rdinv = nc.dram_tensor("rdinv", [1, NCHUNK * 128], f32, kind="ExternalInput")
    w1 = nc.dram_tensor("w1", [D, D], f32, kind="ExternalInput")
    w2 = nc.dram_tensor("w2", [D, D], f32, kind="ExternalInput")
    wh = nc.dram_tensor("wh", [D, DOUT], f32, kind="ExternalInput")
    b1 = nc.dram_tensor("b1", [1, D], f32, kind="ExternalInput")
    b2 = nc.dram_tensor("b2", [1, D], f32, kind="ExternalInput")
    bh = nc.dram_tensor("bh", [1, DOUT], f32, kind="ExternalInput")
    ident = nc.dram_tensor("ident", [128, 128], f32, kind="ExternalInput")
    ones = nc.dram_tensor("ones", [1, 128], f32, kind="ExternalInput")
    out = nc.dram_tensor("out", [G, DOUT], f32, kind="ExternalOutput")

    with tile.TileContext(nc) as tc:
        with tc.tile_pool(name="const", bufs=1) as cp, \
             tc.tile_pool(name="gat", bufs=GATHER_BUFS) as gp, \
             tc.tile_pool(name="bt", bufs=B_BUFS) as bp, \
             tc.tile_pool(name="hs", bufs=4) as hp, \
             tc.tile_pool(name="dg", bufs=4) as dgp, \
             tc.tile_pool(name="agg", bufs=AGG_BUFS, space="PSUM") as aggp, \
             tc.tile_pool(name="hps", bufs=2, space="PSUM") as hpsp, \
             tc.tile_pool(name="pl", bufs=1, space="PSUM") as plp, \
             tc.tile_pool(name="hd", bufs=1, space="PSUM") as hdp, \
             tc.tile_pool(name="dram", bufs=1, space="DRAM") as dp:

            idx_sb = cp.tile([128, itot // 16], i16)
            nc.sync.dma_start(idx_sb[:], idx[:])
            pmat_sb = cp.tile([128, NCHUNK * G], fp8)
            nc.sync.dma_start(pmat_sb[:], pmat[:])
            dinvc_sb = cp.tile([128, NCHUNK], f32)
            nc.sync.dma_start(dinvc_sb[:], dinvc[:])
            dinv2c_sb = cp.tile([128, NCHUNK], f32)
            nc.sync.dma_start(dinv2c_sb[:], dinv2c[:])
            rdinv_sb = cp.tile([1, NCHUNK * 128], f32)
            nc.sync.dma_start(rdinv_sb[:], rdinv[:])
            w1_sb = cp.tile([D, D], f32)
            nc.sync.dma_start(w1_sb[:], w1[:])
            w2_sb = cp.tile([D, D], f32)
            nc.sync.dma_start(w2_sb[:], w2[:])
            wh_sb = cp.tile([D, DOUT], f32)
            nc.sync.dma_start(wh_sb[:], wh[:])
            b1_sb = cp.tile([1, D], f32)
            nc.sync.dma_start(b1_sb[:], b1[:])
            b2_sb = cp.tile([1, D], f32)
            nc.sync.dma_start(b2_sb[:], b2[:])
            bh_sb = cp.tile([1, DOUT], f32)
            nc.sync.dma_start(bh_sb[:], bh[:])
            id_sb = cp.tile([128, 128], f32)
            nc.sync.dma_start(id_sb[:], ident[:])
            ones_sb = cp.tile([1, 128], f32)
            nc.sync.dma_start(ones_sb[:], ones[:])

            h1_bounce = dp.tile([NPC, D], bf16)
            nrep = timed_reps if timed else 1
            h1_fulls = [dp.tile([N, D], bf16, addr_space="Shared",
                                name=f"h1_full_{r}") for r in range(nrep)]
            pool_ins = [dp.tile([G, D], f32, name=f"pool_in_{r}")
                        for r in range(nrep)]
            pool_outs = [dp.tile([G, D], f32, addr_space="Shared",
                                 name=f"pool_out_{r}") for r in range(nrep)]

            # segments grouped by block
            blk_segs = []          # [(b, [(si, g, ni, t0), ...])]
            t = 0
            ipos = 0
            for si, (b, g, ni) in enumerate(seg_tiles):
                if not blk_segs or blk_segs[-1][0] != b:
                    blk_segs.append((b, []))
                blk_segs[-1][1].append((si, g, ni, t, ipos))
                t += -(-ni // 128)
                ipos += ni

            def do_layer(layer, table, slab, w_sb, b_sb, scale_sb):
                for b, segs in blk_segs:
                    bw = BLK if b < NBLK - 1 else NPC - (NBLK - 1) * BLK
                    nck = (bw + 127) // 128
                    agg_ps = aggp.tile([128, BLK], f32, tag="agg")
                    first_mm = True
                    for (si, g, ni, t0, ip0) in segs:
                        nt = -(-ni // 128)
                        gat = gp.tile([128, MAX_SEG_TILES, D], bf16, tag="gat")
                        if ABLATE in ("full", "gather"):
                            nc.gpsimd.dma_gather(
                                gat[:, :nt, :],
                                table[g * GHALF:(g + 1) * GHALF, :],
                                idx_sb[:, ip0 // 16:(ip0 + ni) // 16],
                                ni, ni, D, single_packet=False,
                                queue_num=si % QUEUES)
                        else:
                            r0 = (t0 * 128) % (N - MAX_SEG_TILES * 128)
                            nc.sync.dma_start(
                                gat[:, :nt, :],
                                table[r0:r0 + nt * 128, :].rearrange(
                                    "(a p) d -> p a d", p=128))
                        if ABLATE == "gather":
                            # minimal consumer to keep the pipeline honest
                            nc.tensor.matmul(
                                agg_ps[:, :128], lhsT=gat[:, 0, :],
                                rhs=id_sb[:], start=first_mm, stop=False,
                                skip_group_check=True)
                            first_mm = False
                            continue
                        bseg = bp.tile([128, MAX_SEG_TILES, BLK], fp8, tag="B")
                        nc.sync.dma_start(
                            bseg[:, :nt, :],
                            bmat[:, t0 * BLK:(t0 + nt) * BLK].rearrange(
                                "p (a c) -> p a c", a=nt))
                        for k in range(nt):
                            nc.tensor.matmul(
                                agg_ps[:], lhsT=gat[:, k, :],
                                rhs=bseg[:, k, :],
                                start=first_mm, stop=False,
                                skip_group_check=True)
                            first_mm = False
                    # self-loop diagonal: aggT[:, n] += dinv_n * slab[n]
                    for cki in range(nck):
                        w = min(128, bw - cki * 128)
                        ck = b * (BLK // 128) + cki
                        r0 = b * BLK + cki * 128
                        dloc = dgp.tile([128, D], bf16, tag="dloc")
                        nc.sync.dma_start(dloc[:w, :], slab[r0:r0 + w, :])
                        dscl = dgp.tile([128, D], f32, tag="dscl")
                        nc.scalar.activation(
                            dscl[:w, :], dloc[:w, :],
                            mybir.ActivationFunctionType.Copy)
                        nc.tensor.matmul(
                            agg_ps[:, cki * 128:cki * 128 + w],
                            lhsT=dscl[:w, :], rhs=id_sb[:w, :w],
                            start=False, stop=(cki == nck - 1),
                            skip_group_check=True)
                    if ABLATE == "gather":
                        continue
                    aggt_sb = hp.tile([128, BLK], f32, tag="aggt")
                    nc.scalar.activation(aggt_sb[:], agg_ps[:],
                                         mybir.ActivationFunctionType.Copy)
                    for cki in range(nck):
                        w = min(128, bw - cki * 128)
                        ck = b * (BLK // 128) + cki
                        h_ps = hpsp.tile([128, 128], f32, tag="hps")
                        nc.tensor.matmul(
                            h_ps[:w, :],
                            lhsT=aggt_sb[:, cki * 128:cki * 128 + w],
                            rhs=w_sb[:], start=True, stop=False,
                            skip_group_check=True)
                        nc.tensor.matmul(
                            h_ps[:w, :],
                            lhsT=rdinv_sb[:, ck * 128:ck * 128 + w],
                            rhs=b_sb[:], start=False, stop=True,
                            skip_group_check=True)
                        h_sb = hp.tile([128, 128], bf16, tag="h")
                        nc.scalar.activation(
                            h_sb[:w, :], h_ps[:w, :],
                            mybir.ActivationFunctionType.Relu,
                            scale=scale_sb[:w, ck:ck + 1])
                        r0 = b * BLK + cki * 128
                        if layer == 1:
                            nc.sync.dma_start(
                                h1_bounce[r0:r0 + w, :], h_sb[:w, :])
                        else:
                            nc.tensor.matmul(
                                pool_ps[:], lhsT=pmat_sb[:w, ck * G:(ck + 1) * G],
                                rhs=h_sb[:w, :], start=(ck == 0),
                                stop=(ck == NCHUNK - 1),
                                skip_group_check=True)

            def head(pl_sb):
                pt_ps = hdp.tile([D, G], f32, tag="hd")
                nc.tensor.matmul(pt_ps[:], lhsT=pl_sb[:], rhs=id_sb[:G, :G],
                                 start=True, stop=True, skip_group_check=True)
                pt_sb = hp.tile([D, G], f32, tag="pt")
                nc.vector.tensor_copy(out=pt_sb[:], in_=pt_ps[:])
                lg_ps = hdp.tile([G, DOUT], f32, tag="hd")
                nc.tensor.matmul(lg_ps[:], lhsT=pt_sb[:], rhs=wh_sb[:],
                                 start=True, stop=False)
                nc.tensor.matmul(lg_ps[:], lhsT=ones_sb[:, :G], rhs=bh_sb[:],
                                 start=False, stop=True)
                lg_sb = hp.tile([G, DOUT], f32, tag="lg")
                nc.vector.tensor_copy(out=lg_sb[:], in_=lg_ps[:])
                mx = hp.tile([G, 1], f32, tag="mx")
                nc.vector.reduce_max(mx[:], lg_sb[:], axis=mybir.AxisListType.X)
                nc.vector.tensor_scalar(out=lg_sb[:], in0=lg_sb[:],
                                        scalar1=mx[:], scalar2=None,
                                        op0=mybir.AluOpType.subtract)
                ex = hp.tile([G, DOUT], f32, tag="ex")
                nc.scalar.activation(ex[:], lg_sb[:],
                                     mybir.ActivationFunctionType.Exp)
                sm = hp.tile([G, 1], f32, tag="sm")
                nc.vector.reduce_sum(sm[:], ex[:], axis=mybir.AxisListType.X)
                ls = hp.tile([G, 1], f32, tag="ls")
                nc.scalar.activation(ls[:], sm[:],
                                     mybir.ActivationFunctionType.Ln)
                nc.vector.tensor_scalar(out=lg_sb[:], in0=lg_sb[:],
                                        scalar1=ls[:], scalar2=None,
                                        op0=mybir.AluOpType.subtract)
                nc.sync.dma_start(out[:, :], lg_sb[:])

            def whole(rep):
                do_layer(1, xg, xloc, w1_sb, b1_sb, dinv2c_sb)
                nc.gpsimd.collective_compute(
                    "AllGather", mybir.AluOpType.bypass,
                    replica_groups=[list(range(CORES))],
                    ins=[h1_bounce[:, :].opt()],
                    outs=[h1_fulls[rep][:, :].opt()])
                do_layer(2, h1_fulls[rep], h1_bounce, w2_sb, b2_sb, dinvc_sb)
                pl_sb = hp.tile([G, D], f32, tag="pl")
                nc.scalar.activation(pl_sb[:], pool_ps[:],
                                     mybir.ActivationFunctionType.Copy)
                nc.sync.dma_start(pool_ins[rep][:, :], pl_sb[:])
                nc.gpsimd.collective_compute(
                    "AllReduce", mybir.AluOpType.add,
                    replica_groups=[list(range(CORES))],
                    ins=[pool_ins[rep][:, :].opt()],
                    outs=[pool_outs[rep][:, :].opt()])
                pl2_sb = hp.tile([G, D], f32, tag="pl2")
                nc.sync.dma_start(pl2_sb[:], pool_outs[rep][:, :])
                head(pl2_sb)

            for rep in range(nrep):
                pool_ps = plp.tile([G, D], f32, tag="pool")
                whole(rep)

    nc.compile()
    return nc


def make_in_maps(inputs, per_core, perm, dinv):
    import ml_dtypes
    consts = make_consts()
    x = np.asarray(inputs["x"], dtype=np.float32)
    xt = x * dinv[:, None]                      # dinv in old-id space
    x_perm = np.empty_like(xt)
    x_perm[perm] = xt
    xg16 = np.ascontiguousarray(x_perm.astype(ml_dtypes.bfloat16))
    base = {
        "xg": xg16,
        "w1": np.asarray(inputs["W1"], dtype=np.float32),
        "w2": np.asarray(inputs["W2"], dtype=np.float32),
        "wh": np.asarray(inputs["Wh"], dtype=np.float32),
        "b1": np.asarray(inputs["b1"], dtype=np.float32).reshape(1, D),
        "b2": np.asarray(inputs["b2"], dtype=np.float32).reshape(1, D),
        "bh": np.asarray(inputs["bh"], dtype=np.float32).reshape(1, DOUT),
        **consts,
    }
    in_maps = []
    for c in range(CORES):
        m = dict(base)
        for k in ("idx", "bmat", "pmat", "dinvc", "dinv2c", "rdinv"):
            m[k] = per_core[c][k]
        xl = np.zeros((NCHUNK * 128, D), dtype=ml_dtypes.bfloat16)
        xl[:NPC] = xg16[c * NPC:(c + 1) * NPC]
        m["xloc"] = xl
        in_maps.append(m)
    return in_maps


def kernel(**inputs) -> np.ndarray:
    struct, per_core, perm, dinv = prep(inputs["edge_index"], inputs["batch"])
    nc = build(struct)
    in_maps = make_in_maps(inputs, per_core, perm, dinv)
    from concourse.bass_utils import run_bass_kernel_spmd
    res = run_bass_kernel_spmd(nc, in_maps, core_ids=list(range(CORES)))
    return np.asarray(res.results[0]["out"], dtype=np.float32)


if __name__ == "__main__":
    import reference
    inputs = reference.setup_inputs()
    got = kernel(**{k: np.asarray(v) for k, v in inputs.items()})
    print(got[:2])


# revision 17
# speedup vs baseline: 5.8890x; 3.0057x over previous
"""Trainium2 Bass kernel for ExampleGNN (2-layer GCN + global_add_pool + head).

Self-contained: accepts FULL inputs, shards across 8 NeuronCores internally,
returns the FULL [64, 32] log-softmax output.

Sharding: nodes (and their incident in-edges) are partitioned across 8 cores
with a degree-balancing permutation (node relabeling is internal; pooling is
order-invariant). 128x128 weights replicated. One AllGather shares layer-1
activations between layers; one AllReduce combines pooled partials.

v2 pipeline (per core, per layer) — no DVE in the hot path:
  - norm factored: gather table rows pre-scaled by dinv[src] on host;
    dinv[dst] applied via the post-matmul activation scale (relu(dinv^2 y)),
    bias kept exact with a rdinv-valued K=1 bias matmul
  - one-hot B tiles are therefore pure 0/1: host-precomputed fp8, streamed
    as contiguous DMA, fed straight to PE (lhsT bf16 x rhs fp8)
  - dma_gather pulls scaled h[src] rows (256B bf16) from DRAM, 1024/call,
    round-robin over 4 SWDGE queues
  - PE accumulates aggT[f, n] += gathered^T @ B into PSUM per 256-node block;
    the self-loop diagonal term joins the same PSUM bank via a
    transpose-matmul accumulate (start=False)
  - h chunks: PSUM h = aggT^T W (+ rdinv*b), ACT relu with per-node scale
  - layer 2 pools via host-precomputed fp8 one-hot pmat into persistent PSUM
"""
import numpy as np

import concourse.bacc as bacc
import concourse.mybir as mybir
import concourse.tile as tile

CORES = 8
N = 50000
D = 128
DOUT = 32
G = 64
NPC = N // CORES           # 6250 nodes per core
BLK = 256                  # aggregation block (PSUM free dim)
NBLK = (NPC + BLK - 1) // BLK   # 25 blocks (last has 106 nodes)
NCHUNK = (NPC + 127) // 128     # 49 chunks of 128 nodes
GHALF = 25000              # gather-table split for int16 indices
MAX_SEG_TILES = 8          # cap per dma_gather call (2048-idx calls crash HW)
IDX_ALIGN = 128            # segment row-count alignment (bf16 tables need 128)
ABLATE = "full"            # "gather" | "nogather" | "full"
QUEUES = 4                 # SWDGE queues for parallel dma_gather streams
GATHER_BUFS = 8
B_BUFS = 6
AGG_BUFS = 2

f32 = mybir.dt.float32
bf16 = mybir.dt.bfloat16
fp8 = mybir.dt.bfloat16  # one-hot dtype (bf16: mixed-dtype PE runs at low precision w/ fp8)
i16 = mybir.dt.int16


# ---------------------------------------------------------------- host prep --

def _np_dt(dt):
    return mybir.dt.np(dt)


def _wrap_idxs(idx):
    """[n] -> [128, n//16] int16 wrapped layout (16-partition groups,
    replicated for the 8 gpsimd cores)."""
    n = len(idx)
    t = np.asarray(idx, dtype=np.int16).reshape(n // 16, 16).T
    return np.ascontiguousarray(np.tile(t, (8, 1)))


def prep(edge_index, batch):
    """Host-side index prep. Returns (structure, per_core arrays, node perm,
    dinv in old-id space).

    perm[old_id] = new_id; new ids are contiguous per (core, block) with
    in-degree-balanced assignment (LPT) so per-block edge counts match
    across cores (less tile padding in the shared SPMD program).
    """
    src_o = np.asarray(edge_index[0], dtype=np.int64)
    dst_o = np.asarray(edge_index[1], dtype=np.int64)
    deg = (np.bincount(dst_o, minlength=N) + 1).astype(np.float32)
    dinv = (1.0 / np.sqrt(deg)).astype(np.float32)
    # self-loops handled as a diagonal term (dinv * table row) on the PE side;
    # only real edges go through the gather path

    # ---- LPT balance: assign nodes (by desc in-degree) to 8*NBLK bins
    nbins = CORES * NBLK
    cap = np.full(nbins, BLK, dtype=np.int64)
    cap[NBLK - 1::NBLK] = NPC - (NBLK - 1) * BLK   # last block per core: 106
    order = np.argsort(-deg, kind="stable")
    fill = np.zeros(nbins, dtype=np.int64)
    perm = np.empty(N, dtype=np.int64)
    import heapq
    heap = [(0.0, int(b)) for b in range(nbins)]
    heapq.heapify(heap)
    for nid in order:
        while True:
            l, b = heapq.heappop(heap)
            if fill[b] < cap[b]:
                break
        c, blk_i = divmod(b, NBLK)
        perm[nid] = c * NPC + blk_i * BLK + fill[b]
        fill[b] += 1
        l += float(deg[nid])
        if fill[b] < cap[b]:
            heapq.heappush(heap, (l, b))

    src = perm[src_o]
    dst = perm[dst_o]

    core = dst // NPC
    dstloc = dst - core * NPC
    blk = dstloc // BLK
    dsub = (dstloc % BLK).astype(np.int64)
    grp = (src >= GHALF).astype(np.int64)
    idx16 = (src - grp * GHALF).astype(np.int16)

    so = np.lexsort((grp, blk, core))
    idx_s, core_s = idx16[so], core[so]
    blk_s, grp_s, dsub_s = blk[so], grp[so], dsub[so]

    cnt = np.zeros((CORES, NBLK, 2), dtype=np.int64)
    np.add.at(cnt, (core_s, blk_s, grp_s), 1)
    ccap = -(-cnt.max(axis=0) // IDX_ALIGN) * IDX_ALIGN   # [NBLK, 2]

    seg_tiles = []            # ordered (b, g, nidx); nidx%128==0, <=MAX*128
    for b in range(NBLK):
        for g in range(2):
            r = int(ccap[b, g])
            while r > MAX_SEG_TILES * 128:
                seg_tiles.append((b, g, MAX_SEG_TILES * 128))
                r -= MAX_SEG_TILES * 128
            if r > 0:
                seg_tiles.append((b, g, r))
    ttot = sum(-(-s[2] // 128) for s in seg_tiles)

    starts = np.cumsum(np.concatenate([[0], cnt.reshape(-1)]))[:-1].reshape(cnt.shape)
    itot = sum(s[2] for s in seg_tiles)        # gathered rows (128-aligned)
    fp8np = _np_dt(fp8)
    per_core = []
    for c in range(CORES):
        idx_flat = np.zeros(itot, dtype=np.int16)
        bcol = np.full(ttot * 128, -1, dtype=np.int64)  # -1 = pad row
        pos = 0    # in gathered-row space
        tpos = 0   # in tile space (tile-major rows)
        used = {}
        for (b, g, ni) in seg_tiles:
            u = used.get((b, g), 0)
            take = min(ni, cnt[c, b, g] - u)
            if take > 0:
                sl = slice(starts[c, b, g] + u, starts[c, b, g] + u + take)
                idx_flat[pos:pos + take] = idx_s[sl]
                bcol[tpos:tpos + take] = dsub_s[sl]
                used[(b, g)] = u + take
            pos += ni
            tpos += -(-ni // 128) * 128
        # B one-hot: [ttot*128 rows, BLK] -> [128, ttot*BLK] partition-major
        bm = np.zeros((ttot * 128, BLK), dtype=fp8np)
        rr = np.nonzero(bcol >= 0)[0]
        bm[rr, bcol[rr]] = 1.0
        bm = bm.reshape(ttot, 128, BLK).transpose(1, 0, 2).reshape(128, ttot * BLK)
        per_core.append({
            "idx": _wrap_idxs(idx_flat),
            "bmat": np.ascontiguousarray(bm),
        })

    # per new-node-id vectors
    batch = np.asarray(batch, dtype=np.int64)
    batch_new = np.zeros(N, dtype=np.int64)
    batch_new[perm] = batch
    dinv_new = np.zeros(N, dtype=np.float32)
    dinv_new[perm] = dinv
    for c in range(CORES):
        lo, hi = c * NPC, (c + 1) * NPC
        dv = np.zeros(NCHUNK * 128, dtype=np.float32)
        dv[:NPC] = dinv_new[lo:hi]
        per_core[c]["dinvc"] = np.ascontiguousarray(
            dv.reshape(NCHUNK, 128).T)
        per_core[c]["dinv2c"] = np.ascontiguousarray(
            (dv * dv).reshape(NCHUNK, 128).T)
        rd = np.ones(NCHUNK * 128, dtype=np.float32)
        rd[:NPC] = 1.0 / dinv_new[lo:hi]
        per_core[c]["rdinv"] = rd.reshape(1, NCHUNK * 128)
        pm = np.zeros((NCHUNK * 128, G), dtype=fp8np)
        bl = batch_new[lo:hi]
        pm[np.arange(NPC), bl] = 1.0
        pm = pm.reshape(NCHUNK, 128, G).transpose(1, 0, 2).reshape(128, NCHUNK * G)
        per_core[c]["pmat"] = np.ascontiguousarray(pm)

    struct = {"seg_tiles": seg_tiles, "ttot": ttot, "itot": itot}
    return struct, per_core, perm, dinv


def make_consts():
    ident = np.eye(128, dtype=np.float32)
    ones = np.ones((1, 128), dtype=np.float32)
    return {"ident": ident, "ones": ones}


# ------------------------------------------------------------------ program --

def build(struct, timed_reps=None):
    seg_tiles = struct["seg_tiles"]
    ttot = struct["ttot"]
    itot = struct["itot"]
    timed = timed_reps is not None

    nc = bacc.Bacc("TRN2", target_bir_lowering=False, debug=False,
                   num_devices=CORES, num_swdge_queues=QUEUES)

    xg = nc.dram_tensor("xg", [N, D], bf16, kind="ExternalInput")
    xloc = nc.dram_tensor("xloc", [NCHUNK * 128, D], bf16, kind="ExternalInput")
    idx = nc.dram_tensor("idx", [128, itot // 16], i16, kind="ExternalInput")
    bmat = nc.dram_tensor("bmat", [128, ttot * BLK], fp8, kind="ExternalInput")
    pmat = nc.dram_tensor("pmat", [128, NCHUNK * G], fp8, kind="ExternalInput")
    dinvc = nc.dram_tensor("dinvc", [128, NCHUNK], f32, kind="ExternalInput")
    dinv2c = nc.dram_tensor("dinv2c", [128, NCHUNK], f32, kind="ExternalInput")
    rdinv = nc.dram_tensor("rdinv", [1, NCHUNK * 128], f32, kind="ExternalInput")
    w1 = nc.dram_tensor("w1", [D, D], f32, kind="ExternalInput")
    w2 = nc.dram_tensor("w2", [D, D], f32, kind="ExternalInput")
    wh = nc.dram_tensor("wh", [D, DOUT], f32, kind="ExternalInput")
    b1 = nc.dram_tensor("b1", [1, D], f32, kind="ExternalInput")
    b2 = nc.dram_tensor("b2", [1, D], f32, kind="ExternalInput")
    bh = nc.dram_tensor("bh", [1, DOUT], f32, kind="ExternalInput")
    ident = nc.dram_tensor("ident", [128, 128], f32, kind="ExternalInput")
    ones = nc.dram_tensor("ones", [1, 128], f32, kind="ExternalInput")
    out = nc.dram_tensor("out", [G, DOUT], f32, kind="ExternalOutput")

    with tile.TileContext(nc) as tc:
        with tc.tile_pool(name="const", bufs=1) as cp, \
             tc.tile_pool(name="gat", bufs=GATHER_BUFS) as gp, \
             tc.tile_pool(name="bt", bufs=B_BUFS) as bp, \
             tc.tile_pool(name="hs", bufs=4) as hp, \
             tc.tile_pool(name="dg", bufs=4) as dgp, \
             tc.tile_pool(name="agg", bufs=AGG_BUFS, space="PSUM") as aggp, \
             tc.tile_pool(name="hps", bufs=2, space="PSUM") as hpsp, \
             tc.tile_pool(name="pl", bufs=1, space="PSUM") as plp, \
             tc.tile_pool(name="hd", bufs=1, space="PSUM") as hdp, \
             tc.tile_pool(name="dram", bufs=1, space="DRAM") as dp:

            idx_sb = cp.tile([128, itot // 16], i16)
            nc.sync.dma_start(idx_sb[:], idx[:])
            pmat_sb = cp.tile([128, NCHUNK * G], fp8)
            nc.sync.dma_start(pmat_sb[:], pmat[:])
            dinvc_sb = cp.tile([128, NCHUNK], f32)
            nc.sync.dma_start(dinvc_sb[:], dinvc[:])
            dinv2c_sb = cp.tile([128, NCHUNK], f32)
            nc.sync.dma_start(dinv2c_sb[:], dinv2c[:])
            rdinv_sb = cp.tile([1, NCHUNK * 128], f32)
            nc.sync.dma_start(rdinv_sb[:], rdinv[:])
            w1_sb = cp.tile([D, D], f32)
            nc.sync.dma_start(w1_sb[:], w1[:])
            w2_sb = cp.tile([D, D], f32)
            nc.sync.dma_start(w2_sb[:], w2[:])
            wh_sb = cp.tile([D, DOUT], f32)
            nc.sync.dma_start(wh_sb[:], wh[:])
            b1_sb = cp.tile([1, D], f32)
            nc.sync.dma_start(b1_sb[:], b1[:])
            b2_sb = cp.tile([1, D], f32)
            nc.sync.dma_start(b2_sb[:], b2[:])
            bh_sb = cp.tile([1, DOUT], f32)
            nc.sync.dma_start(bh_sb[:], bh[:])
            id_sb = cp.tile([128, 128], f32)
            nc.sync.dma_start(id_sb[:], ident[:])
            ones_sb = cp.tile([1, 128], f32)
            nc.sync.dma_start(ones_sb[:], ones[:])

            h1_bounce = dp.tile([NPC, D], bf16)
            nrep = timed_reps if timed else 1
            h1_fulls = [dp.tile([N, D], bf16, addr_space="Shared",
                                name=f"h1_full_{r}") for r in range(nrep)]
            pool_ins = [dp.tile([G, D], f32, name=f"pool_in_{r}")
                        for r in range(nrep)]
            pool_outs = [dp.tile([G, D], f32, addr_space="Shared",
                                 name=f"pool_out_{r}") for r in range(nrep)]

            # segments grouped by block
            blk_segs = []          # [(b, [(si, g, ni, t0), ...])]
            t = 0
            ipos = 0
            for si, (b, g, ni) in enumerate(seg_tiles):
                if not blk_segs or blk_segs[-1][0] != b:
                    blk_segs.append((b, []))
                blk_segs[-1][1].append((si, g, ni, t, ipos))
                t += -(-ni // 128)
                ipos += ni

            def do_layer(layer, table, slab, w_sb, b_sb, scale_sb):
                for b, segs in blk_segs:
                    bw = BLK if b < NBLK - 1 else NPC - (NBLK - 1) * BLK
                    nck = (bw + 127) // 128
                    agg_ps = aggp.tile([128, BLK], f32, tag="agg")
                    first_mm = True
                    for (si, g, ni, t0, ip0) in segs:
                        nt = -(-ni // 128)
                        gat = gp.tile([128, MAX_SEG_TILES, D], bf16, tag="gat")
                        if ABLATE in ("full", "gather"):
                            nc.gpsimd.dma_gather(
                                gat[:, :nt, :],
                                table[g * GHALF:(g + 1) * GHALF, :],
                                idx_sb[:, ip0 // 16:(ip0 + ni) // 16],
                                ni, ni, D, single_packet=False,
                                queue_num=si % QUEUES)
                        else:
                            r0 = (t0 * 128) % (N - MAX_SEG_TILES * 128)
                            nc.sync.dma_start(
                                gat[:, :nt, :],
                                table[r0:r0 + nt * 128, :].rearrange(
                                    "(a p) d -> p a d", p=128))
                        if ABLATE == "gather":
                            # minimal consumer to keep the pipeline honest
                            nc.tensor.matmul(
                                agg_ps[:, :128], lhsT=gat[:, 0, :],
                                rhs=gat[:, 0, :], start=first_mm, stop=False,
                                skip_group_check=True)
                            first_mm = False
                            continue
                        bseg = bp.tile([128, MAX_SEG_TILES, BLK], fp8, tag="B")
                        nc.sync.dma_start(
                            bseg[:, :nt, :],
                            bmat[:, t0 * BLK:(t0 + nt) * BLK].rearrange(
                                "p (a c) -> p a c", a=nt))
                        for k in range(nt):
                            nc.tensor.matmul(
                                agg_ps[:], lhsT=gat[:, k, :],
                                rhs=bseg[:, k, :],
                                start=first_mm, stop=False,
                                skip_group_check=True)
                            first_mm = False
                    # self-loop diagonal: aggT[:, n] += dinv_n * slab[n]
                    for cki in range(nck):
                        w = min(128, bw - cki * 128)
                        ck = b * (BLK // 128) + cki
                        r0 = b * BLK + cki * 128
                        dloc = dgp.tile([128, D], bf16, tag="dloc")
                        nc.sync.dma_start(dloc[:w, :], slab[r0:r0 + w, :])
                        dscl = dgp.tile([128, D], f32, tag="dscl")
                        nc.scalar.activation(
                            dscl[:w, :], dloc[:w, :],
                            mybir.ActivationFunctionType.Copy)
                        nc.tensor.matmul(
                            agg_ps[:, cki * 128:cki * 128 + w],
                            lhsT=dscl[:w, :], rhs=id_sb[:w, :w],
                            start=False, stop=(cki == nck - 1),
                            skip_group_check=True)
                    if ABLATE == "gather":
                        continue
                    aggt_sb = hp.tile([128, BLK], f32, tag="aggt")
                    nc.scalar.activation(aggt_sb[:], agg_ps[:],
                                         mybir.ActivationFunctionType.Copy)
                    for cki in range(nck):
                        w = min(128, bw - cki * 128)
                        ck = b * (BLK // 128) + cki
                        h_ps = hpsp.tile([128, 128], f32, tag="hps")
                        nc.tensor.matmul(
                            h_ps[:w, :],
                            lhsT=aggt_sb[:, cki * 128:cki * 128 + w],
                            rhs=w_sb[:], start=True, stop=False,
                            skip_group_check=True)
                        nc.tensor.matmul(
                            h_ps[:w, :],
                            lhsT=rdinv_sb[:, ck * 128:ck * 128 + w],
                            rhs=b_sb[:], start=False, stop=True,
                            skip_group_check=True)
                        h_sb = hp.tile([128, 128], bf16, tag="h")
                        nc.scalar.activation(
                            h_sb[:w, :], h_ps[:w, :],
                            mybir.ActivationFunctionType.Relu,
                            scale=scale_sb[:w, ck:ck + 1])
                        r0 = b * BLK + cki * 128
                        if layer == 1:
                            nc.sync.dma_start(
                                h1_bounce[r0:r0 + w, :], h_sb[:w, :])
                        else:
                            nc.tensor.matmul(
                                pool_ps[:], lhsT=pmat_sb[:w, ck * G:(ck + 1) * G],
                                rhs=h_sb[:w, :], start=(ck == 0),
                                stop=(ck == NCHUNK - 1),
                                skip_group_check=True)

            def head(pl_sb):
                pt_ps = hdp.tile([D, G], f32, tag="hd")
                nc.tensor.matmul(pt_ps[:], lhsT=pl_sb[:], rhs=id_sb[:G, :G],
                                 start=True, stop=True, skip_group_check=True)
                pt_sb = hp.tile([D, G], f32, tag="pt")
                nc.vector.tensor_copy(out=pt_sb[:], in_=pt_ps[:])
                lg_ps = hdp.tile([G, DOUT], f32, tag="hd")
                nc.tensor.matmul(lg_ps[:], lhsT=pt_sb[:], rhs=wh_sb[:],
                                 start=True, stop=False)
                nc.tensor.matmul(lg_ps[:], lhsT=ones_sb[:, :G], rhs=bh_sb[:],
                                 start=False, stop=True)
                lg_sb = hp.tile([G, DOUT], f32, tag="lg")
                nc.vector.tensor_copy(out=lg_sb[:], in_=lg_ps[:])
                mx = hp.tile([G, 1], f32, tag="mx")
                nc.vector.reduce_max(mx[:], lg_sb[:], axis=mybir.AxisListType.X)
                nc.vector.tensor_scalar(out=lg_sb[:], in0=lg_sb[:],
                                        scalar1=mx[:], scalar2=None,
                                        op0=mybir.AluOpType.subtract)
                ex = hp.tile([G, DOUT], f32, tag="ex")
                nc.scalar.activation(ex[:], lg_sb[:],
                                     mybir.ActivationFunctionType.Exp)
                sm = hp.tile([G, 1], f32, tag="sm")
                nc.vector.reduce_sum(sm[:], ex[:], axis=mybir.AxisListType.X)
                ls = hp.tile([G, 1], f32, tag="ls")
                nc.scalar.activation(ls[:], sm[:],
                                     mybir.ActivationFunctionType.Ln)
                nc.vector.tensor_scalar(out=lg_sb[:], in0=lg_sb[:],
                                        scalar1=ls[:], scalar2=None,
                                        op0=mybir.AluOpType.subtract)
                nc.sync.dma_start(out[:, :], lg_sb[:])

            def whole(rep):
                do_layer(1, xg, xloc, w1_sb, b1_sb, dinv2c_sb)
                nc.gpsimd.collective_compute(
                    "AllGather", mybir.AluOpType.bypass,
                    replica_groups=[list(range(CORES))],
                    ins=[h1_bounce[:, :].opt()],
                    outs=[h1_fulls[rep][:, :].opt()])
                do_layer(2, h1_fulls[rep], h1_bounce, w2_sb, b2_sb, dinvc_sb)
                pl_sb = hp.tile([G, D], f32, tag="pl")
                if ABLATE == "gather":
                    nc.vector.memset(pl_sb[:], 0.0)
                else:
                    nc.scalar.activation(pl_sb[:], pool_ps[:],
                                         mybir.ActivationFunctionType.Copy)
                nc.sync.dma_start(pool_ins[rep][:, :], pl_sb[:])
                nc.gpsimd.collective_compute(
                    "AllReduce", mybir.AluOpType.add,
                    replica_groups=[list(range(CORES))],
                    ins=[pool_ins[rep][:, :].opt()],
                    outs=[pool_outs[rep][:, :].opt()])
                pl2_sb = hp.tile([G, D], f32, tag="pl2")
                nc.sync.dma_start(pl2_sb[:], pool_outs[rep][:, :])
                head(pl2_sb)

            for rep in range(nrep):
                if ABLATE != "gather":
                    pool_ps = plp.tile([G, D], f32, tag="pool")
                whole(rep)

    nc.compile()
    return nc


def make_in_maps(inputs, per_core, perm, dinv):
    import ml_dtypes
    consts = make_consts()
    x = np.asarray(inputs["x"], dtype=np.float32)
    xt = x * dinv[:, None]                      # dinv in old-id space
    x_perm = np.empty_like(xt)
    x_perm[perm] = xt
    xg16 = np.ascontiguousarray(x_perm.astype(ml_dtypes.bfloat16))
    base = {
        "xg": xg16,
        "w1": np.asarray(inputs["W1"], dtype=np.float32),
        "w2": np.asarray(inputs["W2"], dtype=np.float32),
        "wh": np.asarray(inputs["Wh"], dtype=np.float32),
        "b1": np.asarray(inputs["b1"], dtype=np.float32).reshape(1, D),
        "b2": np.asarray(inputs["b2"], dtype=np.float32).reshape(1, D),
        "bh": np.asarray(inputs["bh"], dtype=np.float32).reshape(1, DOUT),
        **consts,
    }
    in_maps = []
    for c in range(CORES):
        m = dict(base)
        for k in ("idx", "bmat", "pmat", "dinvc", "dinv2c", "rdinv"):
            m[k] = per_core[c][k]
        xl = np.zeros((NCHUNK * 128, D), dtype=ml_dtypes.bfloat16)
        xl[:NPC] = xg16[c * NPC:(c + 1) * NPC]
        m["xloc"] = xl
        in_maps.append(m)
    return in_maps


def kernel(**inputs) -> np.ndarray:
    struct, per_core, perm, dinv = prep(inputs["edge_index"], inputs["batch"])
    nc = build(struct)
    in_maps = make_in_maps(inputs, per_core, perm, dinv)
    from concourse.bass_utils import run_bass_kernel_spmd
    res = run_bass_kernel_spmd(nc, in_maps, core_ids=list(range(CORES)))
    return np.asarray(res.results[0]["out"], dtype=np.float32)


if __name__ == "__main__":
    import reference
    inputs = reference.setup_inputs()
    got = kernel(**{k: np.asarray(v) for k, v in inputs.items()})
    print(got[:2])
